# revision 24
# baseline (speedup 1.0000x reference)
"""Trainium2 Bass kernel for nn_Attention2 — batch-pair restructure.

Reference computation (per batch):
  q = (hidden @ Wq.T + bq) * scale              -> [S, D] viewed [H, S, HD]
  for each depthwise conv branch b in {1x1, 3x3, 5x5} (VALID padding):
    xb = concat([cls, conv_b(patches)])         -> [L_b, D]
    k = xb @ Wk.T + bk ; v = xb @ Wv.T + bv
    ctx += softmax(q k^T) @ v
  out = (ctx / 3) @ Wo.T + bo

Sharding: data-parallel over batch, 8 batches per core on 8 cores.

Key structure (vs the v1 kernel):
  - Batches processed in PAIRS; matmul outputs for the two batches land in
    the two banks of a [128, 2, 512] PSUM tile so every post-op (exp, bias
    add, copy) covers both batches in ONE instruction (half the instruction
    count on ACT/DVE).
  - All PE operands are fp16 (1 cycle/row unconditionally; weights are
    pre-scaled on host: Wq*=scale, bq*=scale, Wo/=3).
  - Depthwise convs are chained per-partition-scalar FMAs on DVE
    (scalar_tensor_tensor), reading the fp16 transposed image directly and
    writing xb fp16 — no product/reduce tensors, no ACT copies.
  - softmax denominator via an extra ones-column appended per head to V
    (65-column head layout), ctx natural-layout with per-q reciprocal
    normalize; ctx accumulated per 3-head x 2-batch group in a
    [128, 3, 512] PSUM tile.
  - Transposes packed 4-6 per PSUM bank with one copy per bank (epilogue
    transposes in fp16: 1 cy/row and 2x-rate copies).
"""
import math
from contextlib import ExitStack

import numpy as np

import concourse.bass as bass
import concourse.mybir as mybir
import concourse.tile as tile
from concourse import bacc
from concourse.masks import make_identity

B, S, D = 64, 257, 768
H, HD = 12, 64
NT = D // 128          # 6 channel tiles
N_CORES = 8
BPC = B // N_CORES     # batches per core
NP = BPC // 2          # batch pairs per core
SCALE = 1.0 / math.sqrt(HD)

# branch geometry: (kernel_size, out_spatial, seq_len)
BRANCHES = [(1, 16, 257), (3, 14, 197), (5, 12, 145)]
SP = S + 1              # even moving dim

f32 = mybir.dt.float32
f16 = mybir.dt.float16

AF = mybir.ActivationFunctionType
ALU = mybir.AluOpType


def chunks(L, step=128):
    return [(i, min(step, L - i)) for i in range(0, L, step)]


def bcast_free(ap, n):
    """Append a 0-stride free dim of size n to an AP."""
    return bass.AP(tensor=ap.tensor, offset=ap.offset, ap=[*ap.ap, [0, n]])


def ins_b(ap, nb=2):
    """Insert a 0-stride batch dim right after the partition dim."""
    return bass.AP(tensor=ap.tensor, offset=ap.offset,
                   ap=[ap.ap[0], [0, nb], *ap.ap[1:]])


def build(nbatch=BPC, reps=1):
    assert nbatch % 2 == 0
    npair = nbatch // 2
    nc = bacc.Bacc("TRN2", target_bir_lowering=False, debug=False)

    hid = nc.dram_tensor("hidden", [nbatch, S, D], f32, kind="ExternalInput")
    Wq = nc.dram_tensor("Wq", [D, D], f32, kind="ExternalInput")
    Wk = nc.dram_tensor("Wk", [D, D], f32, kind="ExternalInput")
    Wv = nc.dram_tensor("Wv", [D, D], f32, kind="ExternalInput")
    Wo = nc.dram_tensor("Wo", [D, D], f32, kind="ExternalInput")
    bq = nc.dram_tensor("bq", [D], f32, kind="ExternalInput")
    bk = nc.dram_tensor("bk", [D], f32, kind="ExternalInput")
    bv = nc.dram_tensor("bv", [D], f32, kind="ExternalInput")
    bo = nc.dram_tensor("bo", [D], f32, kind="ExternalInput")
    cw = [nc.dram_tensor(f"cw{i}", [D, k * k], f32, kind="ExternalInput")
          for i, (k, _, _) in enumerate(BRANCHES)]
    out = nc.dram_tensor("out", [nbatch, S, D], f32, kind="ExternalOutput")

    with tile.TileContext(nc) as tc, ExitStack() as ctx:
        persist = ctx.enter_context(tc.tile_pool(name="persist", bufs=1))
        pp = ctx.enter_context(tc.tile_pool(name="pp", bufs=2, space="PSUM"))
        cps_pool = ctx.enter_context(tc.tile_pool(name="cpsp", bufs=1,
                                                  space="PSUM"))
        cls_pool = ctx.enter_context(tc.tile_pool(name="clsp", bufs=1,
                                                  space="PSUM"))
        tr_pool = ctx.enter_context(tc.tile_pool(name="trp", bufs=1,
                                                 space="PSUM"))
        pair_pool = ctx.enter_context(tc.tile_pool(name="pair", bufs=1))
        br_pool = ctx.enter_context(tc.tile_pool(name="branch", bufs=1))
        vn_pool = ctx.enter_context(tc.tile_pool(name="vn", bufs=4))
        probs_pool = ctx.enter_context(tc.tile_pool(name="probs", bufs=8))
        small = ctx.enter_context(tc.tile_pool(name="small", bufs=2))

        # ---- constants / weights (once per kernel) ----
        ident32 = persist.tile([128, 128], f32, tag="ident32")
        make_identity(nc, ident32[:])
        ident16 = persist.tile([128, 128], f16, tag="ident16")
        make_identity(nc, ident16[:])

        def load_wT(w_dram, name):
            wT = persist.tile([128, NT, D], f16, tag=name, name=name)
            stages = []
            for half in range(2):
                wstage = pair_pool.tile([128, 3, D], f32, tag="wstage",
                                        bufs=2, name="wstage")
                nc.sync.dma_start(
                    out=wstage[:],
                    in_=w_dram.ap()[half * 384:(half + 1) * 384, :].rearrange(
                        "(m p) j -> p m j", p=128))
                stages.append(wstage)
            for half in range(2):
                wstage = stages[half]
                for m in range(3):
                    mm = half * 3 + m
                    ps = pp.tile([128, 2, 512], f32, tag="pp", name="wtp")
                    for jt in range(4):
                        nc.tensor.matmul(
                            ps[:, 0, jt * 128:(jt + 1) * 128],
                            wstage[:, m, jt * 128:(jt + 1) * 128],
                            ident32[:], is_transpose=True,
                            skip_group_check=True)
                    for jt in range(4, NT):
                        nc.tensor.matmul(
                            ps[:, 1, (jt - 4) * 128:(jt - 3) * 128],
                            wstage[:, m, jt * 128:(jt + 1) * 128],
                            ident32[:], is_transpose=True,
                            skip_group_check=True)
                    nc.vector.tensor_copy(
                        out=wT[:, 0:4, mm * 128:(mm + 1) * 128],
                        in_=ps[:, 0, :].rearrange("p (j c) -> p j c", c=128))
                    nc.vector.tensor_copy(
                        out=wT[:, 4:6, mm * 128:(mm + 1) * 128],
                        in_=ps[:, 1, 0:256].rearrange("p (j c) -> p j c", c=128))
            return wT

        wqT = load_wT(Wq, "wqT")   # host pre-scaled by SCALE
        wkT = load_wT(Wk, "wkT")
        wvT = load_wT(Wv, "wvT")
        woT = load_wT(Wo, "woT")   # host pre-scaled by 1/3

        def load_bT(b_dram, name):
            t = persist.tile([128, NT], f32, tag=name, name=name)
            nc.sync.dma_start(out=t[:], in_=b_dram.ap().rearrange("(m p) -> p m", p=128))
            return t

        bqT = load_bT(bq, "bqT")   # host pre-scaled by SCALE
        bkT = load_bT(bk, "bkT")

        # V/O biases as 1-partition matmul rows: ones stationary + f16 rows
        ones1 = persist.tile([1, 128], f16, tag="ones1")
        nc.gpsimd.memset(ones1[:], 1.0)

        def load_brow(b_dram, name):
            r32 = pair_pool.tile([1, D], f32, tag="wstage", bufs=2,
                                 name="brow32")
            nc.sync.dma_start(out=r32[:],
                              in_=b_dram.ap().rearrange("(o c) -> o c", o=1))
            r16 = persist.tile([1, D], f16, tag=name, name=name)
            nc.vector.tensor_copy(out=r16[:], in_=r32[:])
            return r16

        bvrow = load_brow(bv, "bvrow")
        borow = load_brow(bo, "borow")

        cwt = []
        for i, (k, _, _) in enumerate(BRANCHES):
            t = persist.tile([128, NT, k * k], f32, tag=f"cw{i}", name="cwt")
            nc.sync.dma_start(out=t[:], in_=cw[i].ap().rearrange("(m p) t -> p m t", p=128))
            cwt.append(t)
        cw5h = persist.tile([128, NT, 25], f16, tag="cw5h")
        nc.vector.tensor_copy(out=cw5h[:], in_=cwt[2][:])

        # ---- per-pair pipeline ----

        def stage_load(p):
            """Generator: yields after each emission slice; final value via
            st_load[p]."""
            hn2 = pair_pool.tile([128, 2, 3, D], f32, tag="hn2", bufs=1,
                                 name="hn2")
            for b in range(2):
                nc.sync.dma_start(
                    out=hn2[:, b, 0:2, :],
                    in_=hid.ap()[2 * p + b, 0:256, :].rearrange(
                        "(tt q) c -> q tt c", q=128))
            nc.sync.dma_start(
                out=hn2[0:1, :, 2, :],
                in_=hid.ap()[2 * p:2 * p + 2, 256:257, :].rearrange(
                    "b s c -> s b c"))
            yield

            hT2 = pair_pool.tile([128, NT, 2, SP], f16, tag="hT2", bufs=2,
                                 name="hT2")
            st_load[p] = hT2
            for jt in range(0, NT, 2):
                ps = tr_pool.tile([128, 512], f32, tag="tr", name="trps")
                for b in range(2):
                    for tt in range(2):
                        i = 2 * b + tt
                        nc.tensor.matmul(
                            ps[:, i * 128:(i + 1) * 128],
                            hn2[:, b, tt, jt * 128:(jt + 1) * 128],
                            ident32[:], is_transpose=True,
                            skip_group_check=True)
                nc.scalar.copy(
                    out=hT2[:, jt, :, 0:256],
                    in_=ps[:].rearrange("p (b c) -> p b c", b=2))
                yield
                ps2 = tr_pool.tile([128, 512], f32, tag="tr", name="trps")
                jn = jt + 1
                for b in range(2):
                    for tt in range(2):
                        i = 2 * b + tt
                        nc.tensor.matmul(
                            ps2[:, i * 128:(i + 1) * 128],
                            hn2[:, b, tt, jn * 128:(jn + 1) * 128],
                            ident32[:], is_transpose=True,
                            skip_group_check=True)
                nc.scalar.copy(
                    out=hT2[:, jn, :, 0:256],
                    in_=ps2[:].rearrange("p (b c) -> p b c", b=2))
                yield
            # cls tokens (token 256) for all jt in one round
            ps3 = tr_pool.tile([128, 512], f32, tag="tr", name="trps")
            for jt in range(NT):
                for b in range(2):
                    nc.tensor.matmul(
                        ps3[:, jt * 2 + b:jt * 2 + b + 1],
                        hn2[0:1, b, 2, jt * 128:(jt + 1) * 128],
                        ident32[0:1, 0:1], is_transpose=True,
                        skip_group_check=True)
            nc.scalar.copy(
                out=hT2[:, :, :, 256:257],
                in_=bcast_free(ps3[:, 0:12].rearrange("p (j b) -> p j b", b=2),
                               1))
            yield
            nc.gpsimd.memset(hT2[:, :, :, 257:258], 0.0)

        def stage_q(p, hT2):
            qT2 = pair_pool.tile([128, NT, 2, SP], f16, tag="qT2", bufs=2,
                                 name="qT2")
            st_q[p] = qT2
            for it in range(NT):
                ps = pp.tile([128, 2, 512], f32, tag="pp", name="qps")
                for jt in range(NT):
                    for b in range(2):
                        nc.tensor.matmul(ps[:, b, 0:SP],
                                         wqT[:, jt, it * 128:(it + 1) * 128],
                                         hT2[:, jt, b, :], start=(jt == 0),
                                         stop=(jt == NT - 1),
                                         skip_group_check=True)
                nc.scalar.activation(out=qT2[:, it, :, 0:S],
                                     in_=ps[:, :, 0:S], func=AF.Identity,
                                     bias=bqT[:, it:it + 1])
                yield
            nc.gpsimd.memset(qT2[:, :, :, 257:258], 0.0)

        def stage_xb(p, bi, hT2, xbs):
            ksize, osp, L = BRANCHES[bi]
            Lp = L + 1
            osq = osp * osp
            xbT2 = br_pool.tile([128, NT, 2, SP], f16, tag="xbT", bufs=3,
                                name="xbT2")
            xbs.append(xbT2)
            nc.scalar.copy(out=xbT2[:, :, :, 0:1],
                           in_=hT2[:, :, :, 0:1])
            nc.gpsimd.memset(xbT2[:, :, :, L:Lp], 0.0)
            if ksize == 1:
                for jt in range(NT):
                    wb = bass.AP(tensor=cwt[0].tensor,
                                 offset=cwt[0].offset + jt,
                                 ap=[cwt[0].ap[0], [0, 2], [0, 256]])
                    nc.gpsimd.tensor_tensor(
                        out=xbT2[:, jt, :, 1:257],
                        in0=hT2[:, jt, :, 1:257], in1=wb, op=ALU.mult)
                    if jt % 3 == 2:
                        yield
                return
            ntap = ksize * ksize
            if ksize == 5:
                # fp16 TT products + pairwise in-place add tree: both run at
                # the 2x DVE rate (TensorScalarPtr chains are stuck at 1x)
                prod = br_pool.tile([128, osq * ntap], f16, tag="prod",
                                    bufs=1, name="prod")

                def psl(t0, n):
                    return bass.AP(tensor=prod.tensor,
                                   offset=prod.offset + t0,
                                   ap=[prod.ap[0], [ntap, osq], [1, n]])

                for jt in range(NT):
                    for b in range(2):
                        for dy in range(5):
                            off = jt * (2 * SP) + b * SP + 1 + dy * 16
                            win5 = bass.AP(tensor=hT2.tensor,
                                           offset=hT2.offset + off,
                                           ap=[hT2.ap[0], [16, osp],
                                               [1, osp], [1, 5]])
                            wb = bass.AP(tensor=cw5h.tensor,
                                         offset=cw5h.offset + jt * ntap
                                         + dy * 5,
                                         ap=[cw5h.ap[0], [0, osp],
                                             [0, osp], [1, 5]])
                            po = bass.AP(tensor=prod.tensor,
                                         offset=prod.offset + dy * 5,
                                         ap=[prod.ap[0], [ntap * osp, osp],
                                             [ntap, osp], [1, 5]])
                            nc.vector.tensor_tensor(out=po, in0=win5,
                                                    in1=wb, op=ALU.mult)
                        n = ntap
                        while n > 2:
                            if n & 1:
                                nc.vector.tensor_tensor(
                                    out=psl(0, 1), in0=psl(0, 1),
                                    in1=psl(n - 1, 1), op=ALU.add)
                                n -= 1
                                if n == 2:
                                    break
                            h = n // 2
                            nc.vector.tensor_tensor(
                                out=psl(0, h), in0=psl(0, h),
                                in1=psl(h, h), op=ALU.add)
                            n = h
                        dest = xbT2[:, jt, b, 1:1 + osq].rearrange(
                            "p (s o) -> p s o", o=1)
                        nc.vector.tensor_tensor(out=dest, in0=psl(0, 1),
                                                in1=psl(1, 1), op=ALU.add)
                        yield
                return
            accA = br_pool.tile([128, 2, 256], f16, tag="accA", bufs=1,
                                name="accA")
            accB = br_pool.tile([128, 2, 256], f16, tag="accB", bufs=1,
                                name="accB")

            def win(jt, b, t):
                # 2-free-dim window AP (TensorScalarPtr allows at most 2)
                dy, dx = divmod(t, ksize)
                off = jt * (2 * SP) + b * SP + 1 + dy * 16 + dx
                return bass.AP(tensor=hT2.tensor, offset=hT2.offset + off,
                               ap=[hT2.ap[0], [16, osp], [1, osp]])

            def accv(t_, b, n):
                return t_[:, b, 0:n * n].rearrange("p (r c) -> p r c", r=n)

            for jt in range(NT):
                for b in range(2):
                    cur, nxt = accA, accB
                    nc.vector.tensor_scalar_mul(out=accv(cur, b, osp),
                                                in0=win(jt, b, 0),
                                                scalar1=cwt[bi][:, jt, 0:1])
                    for t in range(1, ntap):
                        if t == ntap - 1:
                            dest = xbT2[:, jt, b, 1:1 + osq].rearrange(
                                "p (r c) -> p r c", r=osp)
                        else:
                            dest = accv(nxt, b, osp)
                        nc.vector.scalar_tensor_tensor(
                            out=dest, in0=win(jt, b, t),
                            scalar=cwt[bi][:, jt, t:t + 1],
                            in1=accv(cur, b, osp), op0=ALU.mult, op1=ALU.add)
                        cur, nxt = nxt, cur
                        if ksize == 5 and t == 12:
                            yield
                    yield
        def stage_k(p, bi, xbT2):
            ksize, osp, L = BRANCHES[bi]
            Lp = L + 1
            kT2 = br_pool.tile([128, NT, 2, S], f16, tag="kT", bufs=2,
                               name="kT2")
            for it in range(NT):
                ps = pp.tile([128, 2, 512], f32, tag="pp", name="kps")
                for jt in range(NT):
                    for b in range(2):
                        nc.tensor.matmul(ps[:, b, 0:Lp],
                                         wkT[:, jt, it * 128:(it + 1) * 128],
                                         xbT2[:, jt, b, 0:Lp],
                                         start=(jt == 0), stop=(jt == NT - 1),
                                         skip_group_check=True)
                nc.scalar.activation(out=kT2[:, it, :, 0:L],
                                     in_=ps[:, :, 0:L], func=AF.Identity,
                                     bias=bkT[:, it:it + 1])
            return kT2

        def stage_v(p, bi, xbT2, kch):
            vns = []
            for (t0, tsz) in kch:
                vn = vn_pool.tile([128, 2, H * 65], f16, tag="vn", name="vn")
                v65 = vn[0:tsz, :, :].rearrange("p b (h c) -> p b h c", c=65)
                nc.gpsimd.memset(v65[:, :, :, 64:65], 1.0)
                for half in range(2):
                    ps = pp.tile([128, 2, 512], f32, tag="pp", name="vps")
                    for jt in range(NT):
                        for b in range(2):
                            nc.tensor.matmul(
                                ps[0:tsz, b, 0:384],
                                xbT2[:, jt, b, t0:t0 + tsz],
                                wvT[:, jt, half * 384:(half + 1) * 384],
                                start=(jt == 0), stop=False,
                                skip_group_check=True)
                    for b in range(2):
                        nc.tensor.matmul(
                            ps[0:tsz, b, 0:384], ones1[0:1, 0:tsz],
                            bvrow[0:1, half * 384:(half + 1) * 384],
                            start=False, stop=True, skip_group_check=True)
                    nc.scalar.copy(
                        out=v65[:, :, half * 6:(half + 1) * 6, 0:64],
                        in_=ps[0:tsz, :, 0:384].rearrange(
                            "p b (h c) -> p b h c", c=64))
                vns.append(vn)
            return vns

        def branch_attn(p, bi, qT2, kT2, vns, ctx_acc2, bg):
            ksize, osp, L = BRANCHES[bi]
            kch = chunks(L)

            def scores_exp(h):
                jt_h, hp = h // 2, (h % 2) * 64
                pts = []
                for (k0, ksz) in kch:
                    ps = pp.tile([128, 2, 512], f32, tag="pp", name="sps")
                    for b in range(2):
                        nc.tensor.matmul(ps[0:ksz, b, 0:SP],
                                         kT2[hp:hp + 64, jt_h, b, k0:k0 + ksz],
                                         qT2[hp:hp + 64, jt_h, b, :],
                                         start=True, stop=True)
                    pt = probs_pool.tile([128, 2, S], f16, tag="probs",
                                         name="pt")
                    nc.scalar.activation(out=pt[0:ksz, :, :],
                                         in_=ps[0:ksz, :, 0:S], func=AF.Exp)
                    pts.append(pt)
                return pts

            pend = {0: scores_exp(0), 1: scores_exp(1)}
            cps = clsps = None
            for h in range(H):
                g, hh = divmod(h, 3)
                if hh == 0:
                    cps = cps_pool.tile([128, 2, 512], f32, tag="cps",
                                        name="cps")
                    clsps = cls_pool.tile([128, 512], f32, tag="cls",
                                          name="clsps")
                if h + 2 < H:
                    pend[h + 2] = scores_exp(h + 2)
                bg()
                pts = pend.pop(h)
                for b in range(2):
                    c0 = b * 195 + hh * 65
                    for kt, (k0, ksz) in enumerate(kch):
                        st_st = (kt == 0), (kt == len(kch) - 1)
                        for qc, (q0, qsz) in enumerate(chunks(S)[:2]):
                            nc.tensor.matmul(
                                cps[0:qsz, qc, c0:c0 + 65],
                                pts[kt][0:ksz, b, q0:q0 + qsz],
                                vns[kt][0:ksz, b, h * 65:(h + 1) * 65],
                                start=st_st[0], stop=st_st[1],
                                skip_group_check=True)
                        nc.tensor.matmul(
                            clsps[0:1, c0:c0 + 65],
                            pts[kt][0:ksz, b, 256:257],
                            vns[kt][0:ksz, b, h * 65:(h + 1) * 65],
                            start=st_st[0], stop=st_st[1],
                            skip_group_check=True)
                if hh == 2:
                    bg()
                    r = small.tile([128, 3, 2, 3], f32, tag="recip", bufs=2,
                                   name="r")
                    r_in = bass.AP(tensor=cps.tensor, offset=cps.offset + 64,
                                   ap=[cps.ap[0], [512, 2], [195, 2], [65, 3]])
                    nc.vector.reciprocal(out=r[:, 0:2], in_=r_in)
                    rc_in = bass.AP(tensor=clsps.tensor,
                                    offset=clsps.offset + 64,
                                    ap=[[clsps.ap[0][0], 1], [0, 1], [195, 2],
                                        [65, 3]])
                    nc.vector.reciprocal(out=r[0:1, 2:3, :, :], in_=rc_in)
                    for qc, (q0, qsz) in enumerate(chunks(S)):
                        if qc < 2:
                            num = bass.AP(
                                tensor=cps.tensor,
                                offset=cps.offset + qc * 512,
                                ap=[[cps.ap[0][0], qsz], [195, 2], [65, 3],
                                    [1, 64]])
                        else:
                            num = bass.AP(
                                tensor=clsps.tensor, offset=clsps.offset,
                                ap=[[clsps.ap[0][0], 1], [195, 2], [65, 3],
                                    [1, 64]])
                        rb = bcast_free(r[0:qsz, qc, :, :], 64)
                        dest = ctx_acc2[0:qsz, :, qc,
                                        g * 192:(g + 1) * 192].rearrange(
                            "p b (h c) -> p b h c", c=64)
                        if bi == 0:
                            nc.vector.tensor_tensor(out=dest, in0=num, in1=rb,
                                                    op=ALU.mult)
                        else:
                            tmp = small.tile([128, 2, 3, 64], f16, tag="ntmp",
                                             bufs=2, name="tmp")
                            nc.vector.tensor_tensor(out=tmp[0:qsz], in0=num,
                                                    in1=rb, op=ALU.mult)
                            nc.gpsimd.tensor_tensor(out=dest, in0=tmp[0:qsz],
                                                    in1=dest, op=ALU.add)

        def epilogue(p, ctx_acc2):
            ctxT2 = pair_pool.tile([128, NT, 2, S], f16, tag="ctxT", bufs=1,
                                   name="ctxT2")
            for jt in range(NT):
                if jt:
                    yield
                ps = tr_pool.tile([128, 1024], f16, tag="tr", name="etp")
                for b in range(2):
                    for qc in range(2):
                        i = 2 * b + qc
                        nc.tensor.matmul(
                            ps[:, i * 128:(i + 1) * 128],
                            ctx_acc2[:, b, qc, jt * 128:(jt + 1) * 128],
                            ident16[:], is_transpose=True,
                            skip_group_check=True)
                for b in range(2):
                    nc.tensor.matmul(
                        ps[:, 512 + 2 * b:512 + 2 * b + 1],
                        ctx_acc2[0:1, b, 2, jt * 128:(jt + 1) * 128],
                        ident16[0:1, 0:1], is_transpose=True,
                        skip_group_check=True)
                nc.scalar.copy(
                    out=ctxT2[:, jt, :, 0:256],
                    in_=ps[:, 0:512].rearrange("p (b c) -> p b c", b=2))
                nc.scalar.copy(
                    out=ctxT2[:, jt, :, 256:257],
                    in_=ps[:, 512:516].rearrange("p (b c) -> p b c", c=2)
                    [:, :, 0:1])

            outsb2 = pair_pool.tile([128, 2, 3, D], f32, tag="outsb", bufs=1,
                                    name="outsb2")
            for tt, (t0, tsz) in enumerate(chunks(S)):
                for half in range(2):
                    yield
                    ps = pp.tile([128, 2, 512], f32, tag="pp", name="ops")
                    for jt in range(NT):
                        for b in range(2):
                            nc.tensor.matmul(
                                ps[0:tsz, b, 0:384],
                                ctxT2[:, jt, b, t0:t0 + tsz],
                                woT[:, jt, half * 384:(half + 1) * 384],
                                start=(jt == 0), stop=False,
                                skip_group_check=True)
                    for b in range(2):
                        nc.tensor.matmul(
                            ps[0:tsz, b, 0:384], ones1[0:1, 0:tsz],
                            borow[0:1, half * 384:(half + 1) * 384],
                            start=False, stop=True, skip_group_check=True)
                    nc.scalar.copy(
                        out=outsb2[0:tsz, :, tt, half * 384:(half + 1) * 384],
                        in_=ps[0:tsz, :, 0:384])

            for b in range(2):
                nc.sync.dma_start(
                    out=out.ap()[2 * p + b, 0:256, :].rearrange(
                        "(tt q) c -> q tt c", q=128),
                    in_=outsb2[:, b, 0:2, :])
            nc.sync.dma_start(
                out=out.ap()[2 * p:2 * p + 2, 256:257, :].rearrange(
                    "b s c -> s b c"),
                in_=outsb2[0:1, :, 2, :])

        loop_cm = tc.For_i(0, reps, 1) if reps > 1 else None
        if loop_cm is not None:
            loop_cm.__enter__()

        st_load = {}
        st_q = {}
        st_xb = {}
        st_xb2 = {}

        def prologue_gen(p):
            yield from stage_load(p)
            yield from stage_q(p, st_load[p])
            xbs = st_xb.setdefault(p, [])
            for bi in range(2):
                yield from stage_xb(p, bi, st_load[p], xbs)
            # branch-2 conv is deferred into the pair's own background
            # stream (keeps bg() fed through the pair tail / last pair)
            st_xb2[p] = stage_xb(p, 2, st_load[p], xbs)

        def drain(gens):
            for gen in gens:
                for _ in gen:
                    pass

        # pair 0's prologue is emitted eagerly
        drain([prologue_gen(0)])
        prev = None
        for p in range(npair):
            qT2 = st_q.pop(p)
            xbs = st_xb.pop(p)
            st_load.pop(p)
            ctx_acc2 = pair_pool.tile([128, 2, 3, D], f16, tag="ctxacc",
                                      bufs=2, name="ctxacc")
            # background emission: interleave prev pair's epilogue and next
            # pair's prologue into this pair's head loops
            gens = []
            if prev is not None:
                gens.append(epilogue(*prev))
            xb2g = st_xb2.pop(p)
            gens.append(xb2g)
            if p + 1 < npair:
                gens.append(prologue_gen(p + 1))

            rr = [0]

            def bg():
                while gens:
                    gen = gens[rr[0] % len(gens)]
                    try:
                        next(gen)
                        rr[0] += 1
                        return
                    except StopIteration:
                        gens.remove(gen)

            for bi in range(3):
                if bi == 2:
                    # branch-2 conv emission must complete before use
                    for _ in xb2g:
                        pass
                    if xb2g in gens:
                        gens.remove(xb2g)
                kT2 = stage_k(p, bi, xbs[bi])
                vns = stage_v(p, bi, xbs[bi], chunks(BRANCHES[bi][2]))
                branch_attn(p, bi, qT2, kT2, vns, ctx_acc2, bg)
            drain(gens)
            prev = (p, ctx_acc2)
        drain([epilogue(*prev)])
        if loop_cm is not None:
            loop_cm.__exit__(None, None, None)

    nc.compile()
    return nc


_COMPILED = {}


def _get_program(nbatch):
    if nbatch not in _COMPILED:
        _COMPILED[nbatch] = build(nbatch)
    return _COMPILED[nbatch]


def make_in_maps(inputs, nbatch=BPC, n_cores=N_CORES):
    h = np.ascontiguousarray(np.asarray(inputs["hidden_states"], dtype=np.float32))
    common = {
        "Wq": np.ascontiguousarray(np.asarray(inputs["Wq"], np.float32) * SCALE),
        "Wk": np.asarray(inputs["Wk"], np.float32),
        "Wv": np.asarray(inputs["Wv"], np.float32),
        "Wo": np.ascontiguousarray(np.asarray(inputs["Wo"], np.float32) / 3.0),
        "bq": np.ascontiguousarray(np.asarray(inputs["bq"], np.float32) * SCALE),
        "bk": np.asarray(inputs["bk"], np.float32),
        "bv": np.asarray(inputs["bv"], np.float32),
        "bo": np.asarray(inputs["bo"], np.float32),
        "cw0": np.ascontiguousarray(np.asarray(inputs["conv1_w"], np.float32).reshape(D, 1)),
        "cw1": np.ascontiguousarray(np.asarray(inputs["conv2_w"], np.float32).reshape(D, 9)),
        "cw2": np.ascontiguousarray(np.asarray(inputs["conv3_w"], np.float32).reshape(D, 25)),
    }
    in_maps = []
    for c in range(n_cores):
        m = dict(common)
        m["hidden"] = np.ascontiguousarray(h[c * nbatch:(c + 1) * nbatch])
        in_maps.append(m)
    return in_maps


def kernel(**inputs) -> np.ndarray:
    from concourse.bass_utils import run_bass_kernel_spmd
    nc = _get_program(BPC)
    in_maps = make_in_maps(inputs)
    res = run_bass_kernel_spmd(nc, in_maps, list(range(N_CORES)))
    return np.concatenate([res.results[c]["out"] for c in range(N_CORES)], axis=0)
